# revision 26
# baseline (speedup 1.0000x reference)
"""Distributed BatchSpectralLoss kernel for Trainium2 (8 NeuronCores).

Computes sum of top-k squared singular values of x (= top-k eigenvalues of
the Gram matrix G = x^T x) for x of shape (8192, 4096), k small (k=1).

Algorithm — implicit block Krylov on x (G is never formed):
  Host: scale x by 1/sqrt(C) with C = 3*||x||_F^2/N so lamhat_1 = O(1) in
  bf16, and draw `chains` random start blocks Omega [4096, b].
  Device, per core r (bf16 matmuls, fp32 PSUM; r owns 1024 rows of x):
    SBUF-resident x slices: xrT = x[rows_r,:]^T (lhsT for U = x_r @ Y) and
    xrN = x[rows_r,:] (lhsT for the partial Y-update).  All DRAM layouts
    are pi-major so DMA lines are contiguous and wide; x chunk loads are
    striped over two DMA engines, and a short PE warmup burst runs during
    them to beat the HAM cold-clock ramp.
    Per application t (q per chain, chains phase-shifted so one chain's
    matmuls hide the other's AllReduce):
      U_t[rows_r] = x[rows_r,:] @ Y_t           [1024, b]  (stays in SBUF)
      Ypart       = x[rows_r,:]^T @ U_t[rows_r] [4096, b]  (pi-major)
      AllReduce(add, bf16) Ypart -> Y_{t+1} full on every core.
    After the last level one extra U_q = x_r @ Y_q.
    The U_t sequence is a Krylov sequence of H = xhat xhat^T (same nonzero
    eigenvalues as Ghat), and its Gram partials only involve the core's own
    rows: SU[a,bb] = U_a[rows_r]^T U_bb[rows_r] (upper triangle, emitted as
    levels complete so the scheduler can fill collective-wait gaps); host
    sums partials over cores.
  Host: S0 = SU[basis, basis], S1 = SU[basis, basis+1] (since
  U_{t+1} = H U_t); rank-guarded generalized Ritz values theta of (S1, S0);
  lambda = C * theta; answer = sum of top k.
"""

import numpy as np
import ml_dtypes

N_CORES = 8
M_ROWS = 8192
N_DIM = 4096
B_BLOCK = 128
Q_APPS = 4
CHAINS = 2
CLIP_TH = 1e-5
XCHUNKS = 4
WARMUP_MMS = 64

_NC_CACHE: dict = {}


def _build_nc(m_rows, n_dim, b, q, n_cores, chains, enable_asserts=False):
    import concourse.mybir as mybir
    import concourse.tile as tile
    from concourse import bacc
    from contextlib import ExitStack

    P = 128
    mloc = m_rows // n_cores   # 1024 rows of x per core
    ko_u = n_dim // P          # 32 k-tiles for U-matmul (and Ypart m-tiles)
    ko_p = mloc // P           # 8 k-tiles for Ypart-matmul (and U m-tiles)
    nlev = q + 1               # stored levels U_0..U_q per chain
    nblk = chains * nlev
    nch = XCHUNKS
    kcu = ko_u // nch          # xrT k-tiles per chunk
    kcp = ko_p // nch          # xrN k-tiles per chunk
    bf = mybir.dt.bfloat16
    f32 = mybir.dt.float32

    nc = bacc.Bacc(
        "TRN2",
        target_bir_lowering=False,
        debug=False,
        enable_asserts=enable_asserts,
        num_devices=n_cores,
    )

    xrl = nc.dram_tensor("xrl", [P, ko_u, mloc], bf, kind="ExternalInput")
    xrn = nc.dram_tensor("xrn", [P, ko_p, n_dim], bf, kind="ExternalInput")
    omega_l = [
        nc.dram_tensor(f"omega{c}", [P, ko_u, b], bf, kind="ExternalInput")
        for c in range(chains)
    ]
    p_out = nc.dram_tensor("p_out", [nblk * b, nblk * b], f32, kind="ExternalOutput")

    yp_in = [[nc.dram_tensor(f"ypi_{c}_{t}", [P, ko_u * b], bf) for t in range(q)]
             for c in range(chains)]
    yp_out = [[nc.dram_tensor(f"ypo_{c}_{t}", [P, ko_u * b], bf, addr_space="Shared")
               for t in range(q)] for c in range(chains)]

    rg = [list(range(n_cores))]

    with tile.TileContext(nc) as tc, ExitStack() as ctx:
        xpool = ctx.enter_context(tc.tile_pool(name="xin", bufs=1))
        ypool = ctx.enter_context(tc.tile_pool(name="yfull", bufs=1))
        yppool = ctx.enter_context(tc.tile_pool(name="ypart", bufs=1))
        slpool = ctx.enter_context(tc.tile_pool(name="slices", bufs=1))
        ppool = ctx.enter_context(tc.tile_pool(name="pout", bufs=3))
        # PSUM: 8 banks = chains*3 (application phase) + 2 (SU-forms/warmup)
        pspool = ctx.enter_context(tc.tile_pool(name="ps", bufs=3, space="PSUM"))
        pspool2 = ctx.enter_context(tc.tile_pool(name="psp", bufs=2, space="PSUM"))

        ycur = {}
        for c in range(chains):
            yf = ypool.tile([P, ko_u, b], bf, tag=f"yf{c}")
            nc.gpsimd.dma_start(yf[:], omega_l[c].ap())
            ycur[c] = yf

        # PE warmup burst during the x loads (HAM clock-gate ramp)
        wps = pspool2.tile([b, b], f32, tag="psp")
        for _ in range(WARMUP_MMS):
            nc.tensor.matmul(wps[:], ycur[0][:, 0, :], ycur[0][:, 0, :],
                             start=True, stop=True)

        xr_ch = []
        xn_ch = []
        for i in range(nch):
            t_ = xpool.tile([P, kcu, mloc], bf, tag=f"xr{i}")
            eng = nc.scalar if i < nch // 2 else nc.sync
            eng.dma_start(t_[:], xrl.ap()[:, i * kcu:(i + 1) * kcu, :])
            xr_ch.append(t_)
        for i in range(nch):
            t_ = xpool.tile([P, kcp, n_dim], bf, tag=f"xn{i}")
            eng = nc.scalar if i < nch // 2 else nc.sync
            eng.dma_start(t_[:], xrn.ap()[:, i * kcp:(i + 1) * kcp, :])
            xn_ch.append(t_)

        stored = []
        blocks = [(c, t) for c in range(chains) for t in range(nlev)]
        bidx = {blk: i for i, blk in enumerate(blocks)}
        usl = {}

        def emit_p(z):
            z_shift_only = blocks[z][1] == q
            for w in stored + ([] if z_shift_only else [z]):
                if z_shift_only and blocks[w][1] == q:
                    continue
                a, bb = (w, z) if w < z else (z, w)
                ps = pspool2.tile([b, b], f32, tag="psp")
                ta = usl[blocks[a]]
                tb = usl[blocks[bb]]
                for ko in range(ko_p):
                    nc.tensor.matmul(
                        ps[:], ta[:, ko * b:(ko + 1) * b], tb[:, ko * b:(ko + 1) * b],
                        start=(ko == 0), stop=(ko == ko_p - 1),
                    )
                ob = ppool.tile([b, b], f32, tag="ob")
                nc.vector.tensor_copy(ob[:], ps[:])
                nc.gpsimd.dma_start(
                    p_out.ap()[a * b:(a + 1) * b, bb * b:(bb + 1) * b], ob[:]
                )
            stored.append(z)

        dmae = [nc.sync, nc.scalar]  # per-chain DMA engine

        def u_mm(c, t):
            """usl[(c,t)] = x[rows_r,:] @ Y_t, 4 m-tiles batched per PSUM
            bank (m2-outer: accumulation groups must be contiguous per bank
            region)."""
            us = slpool.tile([P, ko_p * b], bf, tag=f"usl{c}_{t}")
            for g in range(ko_p // 4):
                ps = pspool.tile([P, 4 * b], f32, tag=f"ps{c}")
                for m2 in range(4):
                    mo = g * 4 + m2
                    for ko in range(ko_u):
                        nc.tensor.matmul(
                            ps[:, m2 * b:(m2 + 1) * b],
                            xr_ch[ko // kcu][:, ko % kcu, mo * P:(mo + 1) * P],
                            ycur[c][:, ko, :],
                            start=(ko == 0),
                            stop=(ko == ko_u - 1),
                        )
                nc.vector.tensor_copy(us[:, g * 4 * b:(g + 1) * 4 * b], ps[:])
            usl[(c, t)] = us
            return us

        for t in range(q):
            for c in range(chains):
                eng = dmae[c]
                us = u_mm(c, t)
                # Ypart = x[rows_r,:]^T @ U_t[rows_r]  [4096, b] pi-major
                yp = yppool.tile([P, ko_u * b], bf, tag=f"yp{c}")
                for g in range(ko_u // 4):
                    ps = pspool.tile([P, 4 * b], f32, tag=f"ps{c}")
                    for m2 in range(4):
                        mo = g * 4 + m2
                        for ko in range(ko_p):
                            nc.tensor.matmul(
                                ps[:, m2 * b:(m2 + 1) * b],
                                xn_ch[ko // kcp][:, ko % kcp, mo * P:(mo + 1) * P],
                                us[:, ko * b:(ko + 1) * b],
                                start=(ko == 0),
                                stop=(ko == ko_p - 1),
                            )
                    nc.vector.tensor_copy(yp[:, g * 4 * b:(g + 1) * 4 * b], ps[:])
                hw2 = (ko_u // 2) * b
                eng.dma_start(yp_in[c][t].ap()[:, 0:hw2], yp[:, 0:hw2])
                nc.gpsimd.dma_start(yp_in[c][t].ap()[:, hw2:], yp[:, hw2:])
                nc.gpsimd.collective_compute(
                    "AllReduce", mybir.AluOpType.add, replica_groups=rg,
                    ins=[yp_in[c][t].ap().opt()], outs=[yp_out[c][t].ap().opt()],
                )
                yf = ypool.tile([P, ko_u, b], bf, tag=f"yf{c}")
                eng.dma_start(yf[:, 0:ko_u // 2, :], yp_out[c][t].ap()[:, 0:hw2])
                nc.gpsimd.dma_start(yf[:, ko_u // 2:, :], yp_out[c][t].ap()[:, hw2:])
                ycur[c] = yf
                emit_p(bidx[(c, t)])

        for c in range(chains):
            u_mm(c, q)
            emit_p(bidx[(c, q)])

    nc.compile()
    return nc


def _get_nc(cfg):
    if cfg not in _NC_CACHE:
        _NC_CACHE[cfg] = _build_nc(*cfg)
    return _NC_CACHE[cfg]


def _ritz_topk(S1, S0, k):
    """Top-k generalized eigenvalues of (S1, S0), f64, rank-guarded."""
    S1 = 0.5 * (S1 + S1.T)
    S0 = 0.5 * (S0 + S0.T)
    d = np.sqrt(np.clip(np.diag(S0), 0, None))
    d = np.where(d > 0, d, 1.0)
    dn = 1.0 / d
    S0n = S0 * dn[:, None] * dn[None, :]
    S1n = S1 * dn[:, None] * dn[None, :]
    w0, v0 = np.linalg.eigh(S0n)
    keep = w0 > (w0.max() * CLIP_TH)
    v = v0[:, keep] / np.sqrt(w0[keep])[None, :]
    m = v.T @ S1n @ v
    m = 0.5 * (m + m.T)
    ev = np.linalg.eigvalsh(m)
    ev = np.clip(ev, 0.0, None)
    return np.sort(ev)[::-1][:k]


def _host_solve(results, k, c_scale):
    b = B_BLOCK
    nlev = Q_APPS + 1
    nblk = CHAINS * nlev
    P64 = np.zeros((nblk * b, nblk * b), dtype=np.float64)
    for r in results:
        p = r["p_out"].astype(np.float64)
        for a in range(nblk):
            for bb in range(a, nblk):
                blk = p[a * b:(a + 1) * b, bb * b:(bb + 1) * b]
                P64[a * b:(a + 1) * b, bb * b:(bb + 1) * b] += blk
                if bb != a:
                    P64[bb * b:(bb + 1) * b, a * b:(a + 1) * b] += blk.T
    bas = [c * nlev + t for c in range(CHAINS) for t in range(Q_APPS)]
    rows = np.concatenate([np.arange(a * b, (a + 1) * b) for a in bas])
    cols = np.concatenate([np.arange((a + 1) * b, (a + 2) * b) for a in bas])
    S0 = P64[np.ix_(rows, rows)]
    S1 = P64[np.ix_(rows, cols)]
    thetas = _ritz_topk(S1, S0, k)
    return float(np.sum(c_scale * thetas))


def _pi_major(a):
    """[K, m] -> [128, K//128, m] with out[pi, ko, m] = a[ko*128 + pi, m]."""
    K, m = a.shape
    return np.ascontiguousarray(a.reshape(K // 128, 128, m).transpose(1, 0, 2))


def _make_inputs(x_np, c_scale):
    bfd = ml_dtypes.bfloat16
    mloc = M_ROWS // N_CORES
    b = B_BLOCK
    xs = (x_np.astype(np.float64) / np.sqrt(c_scale)).astype(np.float32)
    xb = xs.astype(bfd)
    rng = np.random.default_rng(12345)
    omegas = [
        rng.standard_normal((N_DIM, b)).astype(np.float32).astype(bfd)
        for _ in range(CHAINS)
    ]
    om_l = [_pi_major(om) for om in omegas]
    in_maps = []
    for r in range(N_CORES):
        xr = xb[r * mloc:(r + 1) * mloc, :]
        m = {
            "xrl": _pi_major(np.ascontiguousarray(xr.T)),
            "xrn": _pi_major(xr),
        }
        for c in range(CHAINS):
            m[f"omega{c}"] = om_l[c]
        in_maps.append(m)
    return in_maps


def _host_fallback(x_np, k_int):
    """Correct-but-slow host path, used only if the device result is bad."""
    x64 = x_np.astype(np.float64)
    blk = max(8, 2 * k_int)
    rng = np.random.default_rng(0)
    v = rng.standard_normal((x64.shape[1], blk))
    v, _ = np.linalg.qr(v)
    for _ in range(200):
        v, _ = np.linalg.qr(x64.T @ (x64 @ v))
    w = x64 @ v
    ev = np.linalg.eigvalsh(w.T @ w)
    return float(np.sum(np.sort(ev)[::-1][:k_int]))


def kernel(x, k):
    from concourse.bass_utils import run_bass_kernel_spmd

    x_np = np.asarray(x, dtype=np.float32)
    k_int = int(np.asarray(k))
    if k_int <= 0:
        return np.asarray(0.0, dtype=np.float32)

    try:
        v = x_np.ravel()
        fro2 = float(np.dot(v, v))
        c_scale = 3.0 * fro2 / N_DIM
        cfg = (M_ROWS, N_DIM, B_BLOCK, Q_APPS, N_CORES, CHAINS)
        nc = _get_nc(cfg)
        in_maps = _make_inputs(x_np, c_scale)
        res = run_bass_kernel_spmd(nc, in_maps, core_ids=list(range(N_CORES)))
        val = _host_solve(res.results, k_int, c_scale)
        if not np.isfinite(val) or val <= 0:
            raise FloatingPointError(f"bad device result {val}")
    except Exception:
        val = _host_fallback(x_np, k_int)
    return np.asarray(val, dtype=np.float32)


# revision 27
# speedup vs baseline: 3.7867x; 3.7867x over previous
"""Distributed BatchSpectralLoss kernel for Trainium2 (8 NeuronCores).

Computes sum of top-k squared singular values of x (= top-k eigenvalues of
the Gram matrix G = x^T x) for x of shape (8192, 4096), k small (k=1).

Algorithm — implicit block Krylov on x (G is never formed):
  Host: scale x by 1/sqrt(C) with C = 3*||x||_F^2/N so lamhat_1 = O(1) in
  bf16, and draw `chains` random start blocks Omega [4096, b].
  Device, per core r (bf16 matmuls, fp32 PSUM; r owns 1024 rows of x):
    SBUF-resident x slices: xrT = x[rows_r,:]^T (lhsT for U = x_r @ Y) and
    xrN = x[rows_r,:] (lhsT for the partial Y-update).  All DRAM layouts
    are pi-major so DMA lines are contiguous and wide; x chunk loads are
    striped over two DMA engines, and a short PE warmup burst runs during
    them to beat the HAM cold-clock ramp.
    Per application t (q per chain, chains phase-shifted so one chain's
    matmuls hide the other's AllReduce):
      U_t[rows_r] = x[rows_r,:] @ Y_t           [1024, b]  (stays in SBUF)
      Ypart       = x[rows_r,:]^T @ U_t[rows_r] [4096, b]  (pi-major)
      AllReduce(add, bf16) Ypart -> Y_{t+1} full on every core.
    After the last level one extra U_q = x_r @ Y_q.
    The U_t sequence is a Krylov sequence of H = xhat xhat^T (same nonzero
    eigenvalues as Ghat), and its Gram partials only involve the core's own
    rows: SU[a,bb] = U_a[rows_r]^T U_bb[rows_r] (upper triangle, emitted as
    levels complete so the scheduler can fill collective-wait gaps); host
    sums partials over cores.
  Host: S0 = SU[basis, basis], S1 = SU[basis, basis+1] (since
  U_{t+1} = H U_t); rank-guarded generalized Ritz values theta of (S1, S0);
  lambda = C * theta; answer = sum of top k.
"""

import numpy as np
import ml_dtypes

N_CORES = 8
M_ROWS = 8192
N_DIM = 4096
B_BLOCK = 128
Q_APPS = 4
CHAINS = 2
CLIP_TH = 1e-5
XCHUNKS = 2
WARMUP_MMS = 64

_NC_CACHE: dict = {}


def _build_nc(m_rows, n_dim, b, q, n_cores, chains, enable_asserts=False):
    import concourse.mybir as mybir
    import concourse.tile as tile
    from concourse import bacc
    from contextlib import ExitStack

    P = 128
    mloc = m_rows // n_cores   # 1024 rows of x per core
    ko_u = n_dim // P          # 32 k-tiles for U-matmul (and Ypart m-tiles)
    ko_p = mloc // P           # 8 k-tiles for Ypart-matmul (and U m-tiles)
    nlev = q + 1               # stored levels U_0..U_q per chain
    nblk = chains * nlev
    nch = XCHUNKS
    kcu = ko_u // nch          # xrT k-tiles per chunk
    kcp = ko_p // nch          # xrN k-tiles per chunk
    bf = mybir.dt.bfloat16
    f32 = mybir.dt.float32

    nc = bacc.Bacc(
        "TRN2",
        target_bir_lowering=False,
        debug=False,
        enable_asserts=enable_asserts,
        num_devices=n_cores,
    )

    xrl = nc.dram_tensor("xrl", [P, ko_u, mloc], bf, kind="ExternalInput")
    xrn = nc.dram_tensor("xrn", [P, ko_p, n_dim], bf, kind="ExternalInput")
    omega_l = [
        nc.dram_tensor(f"omega{c}", [P, ko_u, b], bf, kind="ExternalInput")
        for c in range(chains)
    ]
    p_out = nc.dram_tensor("p_out", [nblk * b, nblk * b], f32, kind="ExternalOutput")

    yp_in = [[nc.dram_tensor(f"ypi_{c}_{t}", [P, ko_u * b], bf) for t in range(q)]
             for c in range(chains)]
    yp_out = [[nc.dram_tensor(f"ypo_{c}_{t}", [P, ko_u * b], bf, addr_space="Shared")
               for t in range(q)] for c in range(chains)]

    rg = [list(range(n_cores))]

    with tile.TileContext(nc) as tc, ExitStack() as ctx:
        xpool = ctx.enter_context(tc.tile_pool(name="xin", bufs=1))
        ypool = ctx.enter_context(tc.tile_pool(name="yfull", bufs=1))
        yppool = ctx.enter_context(tc.tile_pool(name="ypart", bufs=1))
        slpool = ctx.enter_context(tc.tile_pool(name="slices", bufs=1))
        ppool = ctx.enter_context(tc.tile_pool(name="pout", bufs=3))
        # PSUM: 8 banks = chains*3 (application phase) + 2 (SU-forms/warmup)
        pspool = ctx.enter_context(tc.tile_pool(name="ps", bufs=3, space="PSUM"))
        pspool2 = ctx.enter_context(tc.tile_pool(name="psp", bufs=2, space="PSUM"))

        ycur = {}
        for c in range(chains):
            yf = ypool.tile([P, ko_u, b], bf, tag=f"yf{c}")
            nc.gpsimd.dma_start(yf[:], omega_l[c].ap())
            ycur[c] = yf

        # PE warmup burst during the x loads (HAM clock-gate ramp)
        wps = pspool2.tile([b, b], f32, tag="psp")
        for _ in range(WARMUP_MMS):
            nc.tensor.matmul(wps[:], ycur[0][:, 0, :], ycur[0][:, 0, :],
                             start=True, stop=True)

        xr_ch = []
        xn_ch = []
        for i in range(nch):
            t_ = xpool.tile([P, kcu, mloc], bf, tag=f"xr{i}")
            eng = nc.scalar if i < nch // 2 else nc.sync
            eng.dma_start(t_[:], xrl.ap()[:, i * kcu:(i + 1) * kcu, :])
            xr_ch.append(t_)
        for i in range(nch):
            t_ = xpool.tile([P, kcp, n_dim], bf, tag=f"xn{i}")
            eng = nc.scalar if i < nch // 2 else nc.sync
            eng.dma_start(t_[:], xrn.ap()[:, i * kcp:(i + 1) * kcp, :])
            xn_ch.append(t_)

        stored = []
        blocks = [(c, t) for c in range(chains) for t in range(nlev)]
        bidx = {blk: i for i, blk in enumerate(blocks)}
        usl = {}

        def emit_p(z):
            z_shift_only = blocks[z][1] == q
            for w in stored + ([] if z_shift_only else [z]):
                if z_shift_only and blocks[w][1] == q:
                    continue
                a, bb = (w, z) if w < z else (z, w)
                ps = pspool2.tile([b, b], f32, tag="psp")
                ta = usl[blocks[a]]
                tb = usl[blocks[bb]]
                for ko in range(ko_p):
                    nc.tensor.matmul(
                        ps[:], ta[:, ko * b:(ko + 1) * b], tb[:, ko * b:(ko + 1) * b],
                        start=(ko == 0), stop=(ko == ko_p - 1),
                    )
                ob = ppool.tile([b, b], f32, tag="ob")
                nc.vector.tensor_copy(ob[:], ps[:])
                nc.gpsimd.dma_start(
                    p_out.ap()[a * b:(a + 1) * b, bb * b:(bb + 1) * b], ob[:]
                )
            stored.append(z)

        dmae = [nc.sync, nc.scalar]  # per-chain DMA engine

        def u_mm(c, t):
            """usl[(c,t)] = x[rows_r,:] @ Y_t, 4 m-tiles batched per PSUM
            bank (m2-outer: accumulation groups must be contiguous per bank
            region)."""
            us = slpool.tile([P, ko_p * b], bf, tag=f"usl{c}_{t}")
            for g in range(ko_p // 4):
                ps = pspool.tile([P, 4 * b], f32, tag=f"ps{c}")
                for m2 in range(4):
                    mo = g * 4 + m2
                    for ko in range(ko_u):
                        nc.tensor.matmul(
                            ps[:, m2 * b:(m2 + 1) * b],
                            xr_ch[ko // kcu][:, ko % kcu, mo * P:(mo + 1) * P],
                            ycur[c][:, ko, :],
                            start=(ko == 0),
                            stop=(ko == ko_u - 1),
                        )
                nc.vector.tensor_copy(us[:, g * 4 * b:(g + 1) * 4 * b], ps[:])
            usl[(c, t)] = us
            return us

        for t in range(q):
            for c in range(chains):
                eng = dmae[c]
                us = u_mm(c, t)
                # Ypart = x[rows_r,:]^T @ U_t[rows_r]  [4096, b] pi-major
                yp = yppool.tile([P, ko_u * b], bf, tag=f"yp{c}")
                for g in range(ko_u // 4):
                    ps = pspool.tile([P, 4 * b], f32, tag=f"ps{c}")
                    for m2 in range(4):
                        mo = g * 4 + m2
                        for ko in range(ko_p):
                            nc.tensor.matmul(
                                ps[:, m2 * b:(m2 + 1) * b],
                                xn_ch[ko // kcp][:, ko % kcp, mo * P:(mo + 1) * P],
                                us[:, ko * b:(ko + 1) * b],
                                start=(ko == 0),
                                stop=(ko == ko_p - 1),
                            )
                    nc.vector.tensor_copy(yp[:, g * 4 * b:(g + 1) * 4 * b], ps[:])
                hw2 = (ko_u // 2) * b
                eng.dma_start(yp_in[c][t].ap()[:, 0:hw2], yp[:, 0:hw2])
                nc.gpsimd.dma_start(yp_in[c][t].ap()[:, hw2:], yp[:, hw2:])
                nc.gpsimd.collective_compute(
                    "AllReduce", mybir.AluOpType.add, replica_groups=rg,
                    ins=[yp_in[c][t].ap().opt()], outs=[yp_out[c][t].ap().opt()],
                )
                yf = ypool.tile([P, ko_u, b], bf, tag=f"yf{c}")
                eng.dma_start(yf[:, 0:ko_u // 2, :], yp_out[c][t].ap()[:, 0:hw2])
                nc.gpsimd.dma_start(yf[:, ko_u // 2:, :], yp_out[c][t].ap()[:, hw2:])
                ycur[c] = yf
                emit_p(bidx[(c, t)])

        for c in range(chains):
            u_mm(c, q)
            emit_p(bidx[(c, q)])

    nc.compile()
    return nc


def _get_nc(cfg):
    if cfg not in _NC_CACHE:
        _NC_CACHE[cfg] = _build_nc(*cfg)
    return _NC_CACHE[cfg]


def _ritz_topk(S1, S0, k):
    """Top-k generalized eigenvalues of (S1, S0), f64, rank-guarded."""
    S1 = 0.5 * (S1 + S1.T)
    S0 = 0.5 * (S0 + S0.T)
    d = np.sqrt(np.clip(np.diag(S0), 0, None))
    d = np.where(d > 0, d, 1.0)
    dn = 1.0 / d
    S0n = S0 * dn[:, None] * dn[None, :]
    S1n = S1 * dn[:, None] * dn[None, :]
    w0, v0 = np.linalg.eigh(S0n)
    keep = w0 > (w0.max() * CLIP_TH)
    v = v0[:, keep] / np.sqrt(w0[keep])[None, :]
    m = v.T @ S1n @ v
    m = 0.5 * (m + m.T)
    ev = np.linalg.eigvalsh(m)
    ev = np.clip(ev, 0.0, None)
    return np.sort(ev)[::-1][:k]


def _host_solve(results, k, c_scale):
    b = B_BLOCK
    nlev = Q_APPS + 1
    nblk = CHAINS * nlev
    P64 = np.zeros((nblk * b, nblk * b), dtype=np.float64)
    for r in results:
        p = r["p_out"].astype(np.float64)
        for a in range(nblk):
            for bb in range(a, nblk):
                blk = p[a * b:(a + 1) * b, bb * b:(bb + 1) * b]
                P64[a * b:(a + 1) * b, bb * b:(bb + 1) * b] += blk
                if bb != a:
                    P64[bb * b:(bb + 1) * b, a * b:(a + 1) * b] += blk.T
    bas = [c * nlev + t for c in range(CHAINS) for t in range(Q_APPS)]
    rows = np.concatenate([np.arange(a * b, (a + 1) * b) for a in bas])
    cols = np.concatenate([np.arange((a + 1) * b, (a + 2) * b) for a in bas])
    S0 = P64[np.ix_(rows, rows)]
    S1 = P64[np.ix_(rows, cols)]
    thetas = _ritz_topk(S1, S0, k)
    return float(np.sum(c_scale * thetas))


def _pi_major(a):
    """[K, m] -> [128, K//128, m] with out[pi, ko, m] = a[ko*128 + pi, m]."""
    K, m = a.shape
    return np.ascontiguousarray(a.reshape(K // 128, 128, m).transpose(1, 0, 2))


def _make_inputs(x_np, c_scale):
    bfd = ml_dtypes.bfloat16
    mloc = M_ROWS // N_CORES
    b = B_BLOCK
    xs = (x_np.astype(np.float64) / np.sqrt(c_scale)).astype(np.float32)
    xb = xs.astype(bfd)
    rng = np.random.default_rng(12345)
    omegas = [
        rng.standard_normal((N_DIM, b)).astype(np.float32).astype(bfd)
        for _ in range(CHAINS)
    ]
    om_l = [_pi_major(om) for om in omegas]
    in_maps = []
    for r in range(N_CORES):
        xr = xb[r * mloc:(r + 1) * mloc, :]
        m = {
            "xrl": _pi_major(np.ascontiguousarray(xr.T)),
            "xrn": _pi_major(xr),
        }
        for c in range(CHAINS):
            m[f"omega{c}"] = om_l[c]
        in_maps.append(m)
    return in_maps


def _host_fallback(x_np, k_int):
    """Correct-but-slow host path, used only if the device result is bad."""
    x64 = x_np.astype(np.float64)
    blk = max(8, 2 * k_int)
    rng = np.random.default_rng(0)
    v = rng.standard_normal((x64.shape[1], blk))
    v, _ = np.linalg.qr(v)
    for _ in range(200):
        v, _ = np.linalg.qr(x64.T @ (x64 @ v))
    w = x64 @ v
    ev = np.linalg.eigvalsh(w.T @ w)
    return float(np.sum(np.sort(ev)[::-1][:k_int]))


def kernel(x, k):
    from concourse.bass_utils import run_bass_kernel_spmd

    x_np = np.asarray(x, dtype=np.float32)
    k_int = int(np.asarray(k))
    if k_int <= 0:
        return np.asarray(0.0, dtype=np.float32)

    try:
        v = x_np.ravel()
        fro2 = float(np.dot(v, v))
        c_scale = 3.0 * fro2 / N_DIM
        cfg = (M_ROWS, N_DIM, B_BLOCK, Q_APPS, N_CORES, CHAINS)
        nc = _get_nc(cfg)
        in_maps = _make_inputs(x_np, c_scale)
        res = run_bass_kernel_spmd(nc, in_maps, core_ids=list(range(N_CORES)))
        val = _host_solve(res.results, k_int, c_scale)
        if not np.isfinite(val) or val <= 0:
            raise FloatingPointError(f"bad device result {val}")
    except Exception:
        val = _host_fallback(x_np, k_int)
    return np.asarray(val, dtype=np.float32)


# revision 28
# speedup vs baseline: 3.9466x; 1.0422x over previous
"""Distributed BatchSpectralLoss kernel for Trainium2 (8 NeuronCores).

Computes sum of top-k squared singular values of x (= top-k eigenvalues of
the Gram matrix G = x^T x) for x of shape (8192, 4096), k small (k=1).

Algorithm — implicit block Krylov on x (G is never formed):
  Host: scale x by 1/sqrt(C) with C = 3*||x||_F^2/N so lamhat_1 = O(1) in
  bf16, and draw `chains` random start blocks Omega [4096, b].
  Device, per core r (bf16 matmuls, fp32 PSUM; r owns 1024 rows of x):
    SBUF-resident x slices: xrT = x[rows_r,:]^T (lhsT for U = x_r @ Y) and
    xrN = x[rows_r,:] (lhsT for the partial Y-update).  All DRAM layouts
    are pi-major so DMA lines are contiguous and wide; x chunk loads are
    striped over two DMA engines, and a short PE warmup burst runs during
    them to beat the HAM cold-clock ramp.
    Per application t (q per chain, chains phase-shifted so one chain's
    matmuls hide the other's AllReduce):
      U_t[rows_r] = x[rows_r,:] @ Y_t           [1024, b]  (stays in SBUF)
      Ypart       = x[rows_r,:]^T @ U_t[rows_r] [4096, b]  (pi-major)
      AllReduce(add, bf16) Ypart -> Y_{t+1} full on every core.
    After the last level one extra U_q = x_r @ Y_q.
    The U_t sequence is a Krylov sequence of H = xhat xhat^T (same nonzero
    eigenvalues as Ghat), and its Gram partials only involve the core's own
    rows: SU[a,bb] = U_a[rows_r]^T U_bb[rows_r] (upper triangle, emitted as
    levels complete so the scheduler can fill collective-wait gaps); host
    sums partials over cores.
  Host: S0 = SU[basis, basis], S1 = SU[basis, basis+1] (since
  U_{t+1} = H U_t); rank-guarded generalized Ritz values theta of (S1, S0);
  lambda = C * theta; answer = sum of top k.
"""

import numpy as np
import ml_dtypes

N_CORES = 8
M_ROWS = 8192
N_DIM = 4096
B_BLOCK = 128
Q_APPS = 4
CHAINS = 2
CLIP_TH = 1e-5
XCHUNKS = 4
WARMUP_MMS = 64

_NC_CACHE: dict = {}


def _build_nc(m_rows, n_dim, b, q, n_cores, chains, enable_asserts=False):
    import concourse.mybir as mybir
    import concourse.tile as tile
    from concourse import bacc
    from contextlib import ExitStack

    P = 128
    mloc = m_rows // n_cores   # 1024 rows of x per core
    ko_u = n_dim // P          # 32 k-tiles for U-matmul (and Ypart m-tiles)
    ko_p = mloc // P           # 8 k-tiles for Ypart-matmul (and U m-tiles)
    nlev = q + 1               # stored levels U_0..U_q per chain
    nblk = chains * nlev
    nch = XCHUNKS
    kcu = ko_u // nch          # xrT k-tiles per chunk
    kcp = ko_p // nch          # xrN k-tiles per chunk
    bf = mybir.dt.bfloat16
    f32 = mybir.dt.float32

    nc = bacc.Bacc(
        "TRN2",
        target_bir_lowering=False,
        debug=False,
        enable_asserts=enable_asserts,
        num_devices=n_cores,
    )

    xrl = nc.dram_tensor("xrl", [P, ko_u, mloc], bf, kind="ExternalInput")
    xrn = nc.dram_tensor("xrn", [P, ko_p, n_dim], bf, kind="ExternalInput")
    omega_l = [
        nc.dram_tensor(f"omega{c}", [P, ko_u, b], bf, kind="ExternalInput")
        for c in range(chains)
    ]
    p_out = nc.dram_tensor("p_out", [nblk * b, nblk * b], f32, kind="ExternalOutput")

    yp_in = [[nc.dram_tensor(f"ypi_{c}_{t}", [P, ko_u * b], bf) for t in range(q)]
             for c in range(chains)]
    yp_out = [[nc.dram_tensor(f"ypo_{c}_{t}", [P, ko_u * b], bf, addr_space="Shared")
               for t in range(q)] for c in range(chains)]

    rg = [list(range(n_cores))]

    with tile.TileContext(nc) as tc, ExitStack() as ctx:
        xpool = ctx.enter_context(tc.tile_pool(name="xin", bufs=1))
        ypool = ctx.enter_context(tc.tile_pool(name="yfull", bufs=1))
        yppool = ctx.enter_context(tc.tile_pool(name="ypart", bufs=1))
        slpool = ctx.enter_context(tc.tile_pool(name="slices", bufs=1))
        ppool = ctx.enter_context(tc.tile_pool(name="pout", bufs=3))
        # PSUM: 8 banks = chains*3 (application phase) + 2 (SU-forms/warmup)
        pspool = ctx.enter_context(tc.tile_pool(name="ps", bufs=3, space="PSUM"))
        pspool2 = ctx.enter_context(tc.tile_pool(name="psp", bufs=2, space="PSUM"))

        ycur = {}
        for c in range(chains):
            yf = ypool.tile([P, ko_u, b], bf, tag=f"yf{c}")
            nc.gpsimd.dma_start(yf[:], omega_l[c].ap())
            ycur[c] = yf

        # PE warmup burst during the x loads (HAM clock-gate ramp)
        wps = pspool2.tile([b, b], f32, tag="psp")
        for _ in range(WARMUP_MMS):
            nc.tensor.matmul(wps[:], ycur[0][:, 0, :], ycur[0][:, 0, :],
                             start=True, stop=True)

        xr_ch = []
        xn_ch = []
        for i in range(nch):
            t_ = xpool.tile([P, kcu, mloc], bf, tag=f"xr{i}")
            eng = nc.scalar if i < nch // 2 else nc.sync
            eng.dma_start(t_[:], xrl.ap()[:, i * kcu:(i + 1) * kcu, :])
            xr_ch.append(t_)
        for i in range(nch):
            t_ = xpool.tile([P, kcp, n_dim], bf, tag=f"xn{i}")
            eng = nc.scalar if i < nch // 2 else nc.sync
            eng.dma_start(t_[:], xrn.ap()[:, i * kcp:(i + 1) * kcp, :])
            xn_ch.append(t_)

        stored = []
        blocks = [(c, t) for c in range(chains) for t in range(nlev)]
        bidx = {blk: i for i, blk in enumerate(blocks)}
        usl = {}

        def emit_p(z):
            z_shift_only = blocks[z][1] == q
            for w in stored + ([] if z_shift_only else [z]):
                if z_shift_only and blocks[w][1] == q:
                    continue
                a, bb = (w, z) if w < z else (z, w)
                ps = pspool2.tile([b, b], f32, tag="psp")
                ta = usl[blocks[a]]
                tb = usl[blocks[bb]]
                for ko in range(ko_p):
                    nc.tensor.matmul(
                        ps[:], ta[:, ko * b:(ko + 1) * b], tb[:, ko * b:(ko + 1) * b],
                        start=(ko == 0), stop=(ko == ko_p - 1),
                    )
                ob = ppool.tile([b, b], f32, tag="ob")
                nc.vector.tensor_copy(ob[:], ps[:])
                nc.gpsimd.dma_start(
                    p_out.ap()[a * b:(a + 1) * b, bb * b:(bb + 1) * b], ob[:]
                )
            stored.append(z)

        dmae = [nc.sync, nc.scalar]  # per-chain DMA engine

        def u_mm(c, t):
            """usl[(c,t)] = x[rows_r,:] @ Y_t, 4 m-tiles batched per PSUM
            bank (m2-outer: accumulation groups must be contiguous per bank
            region)."""
            us = slpool.tile([P, ko_p * b], bf, tag=f"usl{c}_{t}")
            for g in range(ko_p // 4):
                ps = pspool.tile([P, 4 * b], f32, tag=f"ps{c}")
                for m2 in range(4):
                    mo = g * 4 + m2
                    for ko in range(ko_u):
                        nc.tensor.matmul(
                            ps[:, m2 * b:(m2 + 1) * b],
                            xr_ch[ko // kcu][:, ko % kcu, mo * P:(mo + 1) * P],
                            ycur[c][:, ko, :],
                            start=(ko == 0),
                            stop=(ko == ko_u - 1),
                        )
                nc.vector.tensor_copy(us[:, g * 4 * b:(g + 1) * 4 * b], ps[:])
            usl[(c, t)] = us
            return us

        for t in range(q):
            for c in range(chains):
                eng = dmae[c]
                us = u_mm(c, t)
                # Ypart = x[rows_r,:]^T @ U_t[rows_r]  [4096, b] pi-major
                yp = yppool.tile([P, ko_u * b], bf, tag=f"yp{c}")
                for g in range(ko_u // 4):
                    ps = pspool.tile([P, 4 * b], f32, tag=f"ps{c}")
                    for m2 in range(4):
                        mo = g * 4 + m2
                        for ko in range(ko_p):
                            nc.tensor.matmul(
                                ps[:, m2 * b:(m2 + 1) * b],
                                xn_ch[ko // kcp][:, ko % kcp, mo * P:(mo + 1) * P],
                                us[:, ko * b:(ko + 1) * b],
                                start=(ko == 0),
                                stop=(ko == ko_p - 1),
                            )
                    nc.vector.tensor_copy(yp[:, g * 4 * b:(g + 1) * 4 * b], ps[:])
                hw2 = (ko_u // 2) * b
                eng.dma_start(yp_in[c][t].ap()[:, 0:hw2], yp[:, 0:hw2])
                nc.gpsimd.dma_start(yp_in[c][t].ap()[:, hw2:], yp[:, hw2:])
                nc.gpsimd.collective_compute(
                    "AllReduce", mybir.AluOpType.add, replica_groups=rg,
                    ins=[yp_in[c][t].ap().opt()], outs=[yp_out[c][t].ap().opt()],
                )
                yf = ypool.tile([P, ko_u, b], bf, tag=f"yf{c}")
                eng.dma_start(yf[:, 0:ko_u // 2, :], yp_out[c][t].ap()[:, 0:hw2])
                nc.gpsimd.dma_start(yf[:, ko_u // 2:, :], yp_out[c][t].ap()[:, hw2:])
                ycur[c] = yf
                emit_p(bidx[(c, t)])

        for c in range(chains):
            u_mm(c, q)
            emit_p(bidx[(c, q)])

    nc.compile()
    return nc


def _get_nc(cfg):
    if cfg not in _NC_CACHE:
        _NC_CACHE[cfg] = _build_nc(*cfg)
    return _NC_CACHE[cfg]


def _ritz_topk(S1, S0, k):
    """Top-k generalized eigenvalues of (S1, S0), f64, rank-guarded."""
    S1 = 0.5 * (S1 + S1.T)
    S0 = 0.5 * (S0 + S0.T)
    d = np.sqrt(np.clip(np.diag(S0), 0, None))
    d = np.where(d > 0, d, 1.0)
    dn = 1.0 / d
    S0n = S0 * dn[:, None] * dn[None, :]
    S1n = S1 * dn[:, None] * dn[None, :]
    w0, v0 = np.linalg.eigh(S0n)
    keep = w0 > (w0.max() * CLIP_TH)
    v = v0[:, keep] / np.sqrt(w0[keep])[None, :]
    m = v.T @ S1n @ v
    m = 0.5 * (m + m.T)
    ev = np.linalg.eigvalsh(m)
    ev = np.clip(ev, 0.0, None)
    return np.sort(ev)[::-1][:k]


def _host_solve(results, k, c_scale):
    b = B_BLOCK
    nlev = Q_APPS + 1
    nblk = CHAINS * nlev
    P64 = np.zeros((nblk * b, nblk * b), dtype=np.float64)
    for r in results:
        p = r["p_out"].astype(np.float64)
        for a in range(nblk):
            for bb in range(a, nblk):
                blk = p[a * b:(a + 1) * b, bb * b:(bb + 1) * b]
                P64[a * b:(a + 1) * b, bb * b:(bb + 1) * b] += blk
                if bb != a:
                    P64[bb * b:(bb + 1) * b, a * b:(a + 1) * b] += blk.T
    bas = [c * nlev + t for c in range(CHAINS) for t in range(Q_APPS)]
    rows = np.concatenate([np.arange(a * b, (a + 1) * b) for a in bas])
    cols = np.concatenate([np.arange((a + 1) * b, (a + 2) * b) for a in bas])
    S0 = P64[np.ix_(rows, rows)]
    S1 = P64[np.ix_(rows, cols)]
    thetas = _ritz_topk(S1, S0, k)
    return float(np.sum(c_scale * thetas))


def _pi_major(a):
    """[K, m] -> [128, K//128, m] with out[pi, ko, m] = a[ko*128 + pi, m]."""
    K, m = a.shape
    return np.ascontiguousarray(a.reshape(K // 128, 128, m).transpose(1, 0, 2))


def _make_inputs(x_np, c_scale):
    bfd = ml_dtypes.bfloat16
    mloc = M_ROWS // N_CORES
    b = B_BLOCK
    xs = (x_np.astype(np.float64) / np.sqrt(c_scale)).astype(np.float32)
    xb = xs.astype(bfd)
    rng = np.random.default_rng(12345)
    omegas = [
        rng.standard_normal((N_DIM, b)).astype(np.float32).astype(bfd)
        for _ in range(CHAINS)
    ]
    om_l = [_pi_major(om) for om in omegas]
    in_maps = []
    for r in range(N_CORES):
        xr = xb[r * mloc:(r + 1) * mloc, :]
        m = {
            "xrl": _pi_major(np.ascontiguousarray(xr.T)),
            "xrn": _pi_major(xr),
        }
        for c in range(CHAINS):
            m[f"omega{c}"] = om_l[c]
        in_maps.append(m)
    return in_maps


def _host_fallback(x_np, k_int):
    """Correct-but-slow host path, used only if the device result is bad."""
    x64 = x_np.astype(np.float64)
    blk = max(8, 2 * k_int)
    rng = np.random.default_rng(0)
    v = rng.standard_normal((x64.shape[1], blk))
    v, _ = np.linalg.qr(v)
    for _ in range(200):
        v, _ = np.linalg.qr(x64.T @ (x64 @ v))
    w = x64 @ v
    ev = np.linalg.eigvalsh(w.T @ w)
    return float(np.sum(np.sort(ev)[::-1][:k_int]))


def kernel(x, k):
    from concourse.bass_utils import run_bass_kernel_spmd

    x_np = np.asarray(x, dtype=np.float32)
    k_int = int(np.asarray(k))
    if k_int <= 0:
        return np.asarray(0.0, dtype=np.float32)

    try:
        v = x_np.ravel()
        fro2 = float(np.dot(v, v))
        c_scale = 3.0 * fro2 / N_DIM
        cfg = (M_ROWS, N_DIM, B_BLOCK, Q_APPS, N_CORES, CHAINS)
        nc = _get_nc(cfg)
        in_maps = _make_inputs(x_np, c_scale)
        res = run_bass_kernel_spmd(nc, in_maps, core_ids=list(range(N_CORES)))
        val = _host_solve(res.results, k_int, c_scale)
        if not np.isfinite(val) or val <= 0:
            raise FloatingPointError(f"bad device result {val}")
    except Exception:
        val = _host_fallback(x_np, k_int)
    return np.asarray(val, dtype=np.float32)
